# revision 44
# baseline (speedup 1.0000x reference)
"""Trainium2 Bass kernel for nn_CaptionHead (segment_reduce).

Computes, for full-size inputs:
    point_feats = adapter_feats[v2p_map]            # [N_PTS, D]
    gathered    = point_feats[point_idx]            # [T, D]
    sums        = segment_sum(gathered, seg_ids, S) # [S, D]
    pooled      = l2norm(sums / max(counts, 1))     # == l2norm(sums)
    logits      = (pooled @ l2norm(ce).T) * exp(logit_scale)

Distribution: adapter_feats is sharded by voxel across the 8 cores
(25000 rows each, stored bf16 so a gathered row is one full-rate 512 B
descriptor; shard-local indices fit the int16 dma_gather path).  Host
preprocessing composes cidx = v2p_map[point_idx], routes each point to
the core owning its voxel, and DEDUPLICATES per (core, 128-seg chunk,
voxel): the gather is descriptor-count-bound on real HW (~2.3 ns/row
fixed cost), so each distinct row is fetched once and scattered with
count-weighted one-hot layers (layer L = the L-th (seg, count) pair of
a row; rows sort multiplicity-first so layers L>=2 and count>1 tiles
stay a short prefix).

On device, each core streams its rows with dma_gather (4 SWDGE queues,
a shared full-batch count register -- per-batch Q7 reg_loads were a
serial bottleneck -- and a 13-buffer pipeline; PAD INDICES MUST BE -1:
non-negative pads hang the gather ucode).  One-hot weights are built on
the DVE in batches of 8 tiles in a transposed [128, seg, tile] layout
against a pre-expanded iota so both streamed operands keep a packed
2-byte last dim (the DVE 2x fast path; a per-tile build was ~3x
slower).  Count-weighted prefix tiles use the fused TensorScalarPtr
(iota == seg) * cnt path.  Each chunk accumulates its [128, 256] PSUM
block via the one-hot matmuls, the [S, D] partial sums ReduceScatter
(f32 -- bf16 collectives hang) in two halves overlapped with the loop,
and each core normalizes its 256 segment rows (1/count cancels in the
l2norm) and multiplies against host-prenormalized caption embeddings
(bf16).  Core r returns logits rows for chunks r and 8+r; the host
concatenates the blocks.
"""

import math

import numpy as np

N_VOX = 200000
N_PTS = 500000
T_FULL = 1000000
S_FULL = 2048
D_FULL = 256
N_CORES = 8
P = 128


def _preprocess(v2p_map, point_idx, seg_ids, n_cores, vox_per_core, n_chunks, trim=True):
    """Route points to voxel-owning cores, dedup per (core, chunk, voxel), pad.

    Each distinct (core, chunk, voxel) becomes ONE gathered row (the gather is
    descriptor-count-bound on HW, so duplicate rows are pure waste).  A row
    scatters into its chunk's 128 segments through count-weighted one-hot
    layers: layer L holds the L-th (seg, count) pair of the row.  Rows are
    ordered within each chunk by pair-count descending so layer L>=2 only
    touches the first few tiles.

    Returns (idx16, layers, tiles_per_chunk, layer_tiles, counts):
      idx16[m]:  [128, NIDX//16] int16 shard-local voxel index per row in
                 dma_gather's 16-partition-wrapped, 8x-replicated layout.
      layers[m]: list over L of (segf, cntf) float32 [128, n_chunks*layer_tiles[L]]
                 chunk-local seg id / multiplicity (seg -1, cnt 0 for padding).
      counts:    [n_cores, n_chunks] valid row count per cell (for trim).
    """
    v2p = np.asarray(v2p_map).astype(np.int64)
    pidx = np.asarray(point_idx).astype(np.int64)
    seg = np.asarray(seg_ids).astype(np.int64)
    cidx = v2p[pidx]                      # composed voxel index per point
    core = cidx // vox_per_core
    lvox = cidx - core * vox_per_core
    chunk = seg >> 7                      # 128 segments per chunk
    segl = seg & 127
    cell = core * n_chunks + chunk        # [0, n_cores*n_chunks)
    key3 = (cell * vox_per_core + lvox) * 128 + segl
    uk, ucnt = np.unique(key3, return_counts=True)      # sorted pairs
    rowkey = uk // 128                                  # (cell, voxel)
    useg = (uk % 128).astype(np.float32)
    urow, first_idx, row_inv, npairs = np.unique(
        rowkey, return_index=True, return_inverse=True, return_counts=True
    )
    pair_layer = np.arange(len(uk)) - first_idx[row_inv]  # rank within row
    rcell = (urow // vox_per_core).astype(np.int64)
    rvox = (urow % vox_per_core).astype(np.int64)
    # rows with any multiplicity>1 pair first (their one-hots need the
    # slower fused count-scale op), then by pair count so layer L>=2 stays a
    # small tile prefix, then voxel for determinism
    has2 = np.zeros(len(urow), bool)
    np.logical_or.at(has2, row_inv, ucnt > 1)
    order = np.lexsort((rvox, -npairs, ~has2, rcell))
    counts = np.bincount(rcell, minlength=n_cores * n_chunks)
    tiles_per_chunk = max(1, math.ceil(counts.max() / P))
    offs = np.concatenate([[0], np.cumsum(counts)])
    pos_sorted = np.arange(len(urow)) - offs[rcell[order]]
    pos = np.empty(len(urow), np.int64)
    pos[order] = pos_sorted                              # in-cell row slot
    max_layers = int(npairs.max())
    # layer_tiles[L]: tiles covering every pair at layer L (from actual max
    # slot); cnt_tiles[L]: tile prefix containing every multiplicity>1 pair
    pr_cell0 = rcell[row_inv]
    pr_pos0 = pos[row_inv]
    layer_tiles = []
    cnt_tiles = []
    for L in range(max_layers):
        m = pair_layer == L
        layer_tiles.append(int(pr_pos0[m].max()) // P + 1)
        m2 = m & (ucnt > 1)
        cnt_tiles.append(int(pr_pos0[m2].max()) // P + 1 if m2.any() else 0)
    npc = tiles_per_chunk * P
    vox_arr = np.full((n_cores, n_chunks, npc), -1 if trim else 0, np.int16)
    vox_arr.reshape(n_cores * n_chunks, npc)[rcell, pos] = rvox.astype(np.int16)
    seg_arrs, cnt_arrs = [], []
    pr_cell = pr_cell0                                   # per-pair cell
    pr_pos = pr_pos0                                     # per-pair row slot
    for L in range(max_layers):
        w = layer_tiles[L] * P
        sa = np.full((n_cores, n_chunks, w), -1.0, np.float32)
        ca = np.zeros((n_cores, n_chunks, w), np.float32)
        m = pair_layer == L
        sa.reshape(n_cores * n_chunks, w)[pr_cell[m], pr_pos[m]] = useg[m]
        ca.reshape(n_cores * n_chunks, w)[pr_cell[m], pr_pos[m]] = ucnt[m]
        seg_arrs.append(sa)
        cnt_arrs.append(ca)

    idx16 = []
    layers = []
    for m in range(n_cores):
        arr = vox_arr[m].reshape(-1, 16).T          # [16, NIDX//16]
        idx16.append(np.ascontiguousarray(np.tile(arr, (8, 1))))
        layers.append([
            (np.ascontiguousarray(seg_arrs[L][m].reshape(-1, P).T),
             np.ascontiguousarray(cnt_arrs[L][m].reshape(-1, P).T))
            for L in range(max_layers)
        ])
    return (idx16, layers, tiles_per_chunk, layer_tiles, cnt_tiles,
            counts.reshape(n_cores, n_chunks))


def _build_nc(tiles_per_chunk, vox_per_core, D, S, n_cores, layer_tiles,
              cnt_tiles=None, batch_tiles=8, main_reps=1, mode="full",
              single_core=False, gp_bufs=13, oh_bufs=8, acc_bufs=4,
              full_batches=None, need_memset=False):
    """mode: "full" | "nomm" (gathers only) | "nogather" (compute only)
    | "noonehot" (gather + matmul, constant weights).  main_reps repeats the
    main loop; with mode="full" the output stays correct (each rep recomputes
    the same sums; only the last is copied out)."""
    import concourse.bacc as bacc
    import concourse.mybir as mybir
    import concourse.tile as tile
    from concourse.masks import make_identity

    f32 = mybir.dt.float32
    bf16 = mybir.dt.bfloat16
    i16 = mybir.dt.int16
    i32 = mybir.dt.int32
    n_chunks = S // P
    NT = n_chunks * tiles_per_chunk            # total point tiles
    NIDX = NT * P                              # total gathered rows
    out_rows = S // n_cores                    # 256
    blk_tiles = out_rows // P                  # 2
    k_tiles = D // P                           # 2
    n_cols = 512                               # moving-operand tile width
    n_tiles_out = S // n_cols                  # 4

    nc = bacc.Bacc(
        "TRN2",
        target_bir_lowering=False,
        debug=False,
        enable_asserts=False,
        num_devices=n_cores,
        # SWDGE descriptor-ring carveout: must hold two in-flight
        # dma_gathers of batch_tiles*128 descriptors each.
        dynamic_dma_scratch_size=32768,
        # round-robin gathers over all 4 SWDGE queues: each queue's
        # descriptor generation runs on its own Q7 core pair.
        num_swdge_queues=4,
    )

    # adapter rows are plain bf16: 512 B gathered per point, which both
    # halves HBM gather traffic vs an f32/hi-lo row and stays exactly at the
    # DMA full-rate descriptor size (>= 512 B).  Precision: the one-hot
    # matmul accumulates bf16 values in f32 PSUM; per-logit error lands
    # ~1e-3 relative, far under the 2e-2 gate.
    adapter = nc.dram_tensor("adapter", [vox_per_core, D], bf16, kind="ExternalInput")
    idx16 = nc.dram_tensor("idx16", [P, NIDX // 16], i16, kind="ExternalInput")
    # per-layer (seg, count) pairs, concatenated along columns; layer L
    # occupies cols [layer_off[L], layer_off[L+1]) with n_chunks*layer_tiles[L]
    # columns (tile-major within each layer).
    layer_off = [0]
    for lt in layer_tiles:
        layer_off.append(layer_off[-1] + n_chunks * lt)
    if cnt_tiles is None:
        cnt_tiles = [lt for lt in layer_tiles]  # all tiles use the fused op
    segf = nc.dram_tensor("segf", [P, layer_off[-1]], f32, kind="ExternalInput")
    cntf = nc.dram_tensor("cntf", [P, layer_off[-1]], f32, kind="ExternalInput")
    # bf16 copy of segf for the batched transposed one-hot builds, and the
    # batch_tiles-fold expanded iota ([P, j, b] -> j) whose packed last dim
    # keeps the DVE 2x fast path on both streamed operands
    segfb = nc.dram_tensor("segfb", [P, layer_off[-1]], bf16, kind="ExternalInput")
    iota = nc.dram_tensor("iota", [P, P], bf16, kind="ExternalInput")
    iotax = nc.dram_tensor("iotax", [P, P * batch_tiles], bf16, kind="ExternalInput")
    # caption embeds arrive L2-normalized from the host; only the transposed
    # copy is needed for the logits matmul.
    cet = nc.dram_tensor("cet", [D, S], bf16, kind="ExternalInput")
    lsr = nc.dram_tensor("lsr", [P, 1], f32, kind="ExternalInput")
    n_batches = (tiles_per_chunk + batch_tiles - 1) // batch_tiles
    if full_batches is None:
        full_batches = [False] * (n_chunks * n_batches)
    cnts = nc.dram_tensor("cnts", [1, n_chunks * n_batches], i32, kind="ExternalInput")
    out = nc.dram_tensor("logits_block", [out_rows, S], f32, kind="ExternalOutput")
    cc_in = nc.dram_tensor("cc_in", [S, D], f32, kind="Internal")
    half_rows = S // 2
    cc_out_h = [
        nc.dram_tensor(f"cc_out{h}", [half_rows // n_cores, D], f32, kind="Internal")
        for h in range(2)
    ]

    with tile.TileContext(nc) as tc:
        with (
            tc.tile_pool(name="const", bufs=1) as constp,
            tc.tile_pool(name="gather", bufs=gp_bufs) as gp,
            tc.tile_pool(name="oh", bufs=oh_bufs) as ohp,
            tc.tile_pool(name="ohb", bufs=8) as ohbp,
            tc.tile_pool(name="misc", bufs=1) as miscp,
            tc.tile_pool(name="fin", bufs=1) as finp,
            tc.tile_pool(name="fpsum", bufs=1, space="PSUM") as fpp,
        ):
            # ---- prologue loads, shortest-critical-path first ----
            # SP (sync) queue: batch counts + the first idx stripe unblock
            # the first gather within ~2 us; segf/iota unblock the one-hots.
            cnt_sb = constp.tile([1, n_chunks * n_batches], i32)
            nc.sync.dma_start(cnt_sb[:], cnts.ap())
            # idx stripes are separate tiles so a gather only depends on the
            # stripe that covers its chunk (dep tracking is per-tile).
            chunk_cols = tiles_per_chunk * P // 16
            stripe_chunks = max(1, 2048 // chunk_cols)
            stripe_cols = stripe_chunks * chunk_cols
            stripe_bounds = []
            for s0 in range(0, NIDX // 16, stripe_cols):
                stripe_bounds.append((s0, min(s0 + stripe_cols, NIDX // 16)))
            idx_parts = [
                constp.tile([P, s1 - s0], i16, name=f"idx{s0}")
                for s0, s1 in stripe_bounds
            ]
            nc.sync.dma_start(idx_parts[0][:], idx16.ap()[:, : stripe_bounds[0][1]])
            iota_sb = constp.tile([P, P], bf16)
            nc.sync.dma_start(iota_sb[:], iota.ap())
            segf_sb = constp.tile([P, layer_off[-1]], f32)
            nc.sync.dma_start(segf_sb[:], segf.ap())
            cntf_sb = constp.tile([P, layer_off[-1]], f32)
            nc.sync.dma_start(cntf_sb[:], cntf.ap())
            segfb_sb = constp.tile([P, layer_off[-1]], bf16)
            nc.sync.dma_start(segfb_sb[:], segfb.ap())
            iotax_sb = constp.tile([P, P * batch_tiles], bf16)
            nc.sync.dma_start(iotax_sb[:], iotax.ap())
            ls_sb = finp.tile([P, 1], f32)
            nc.sync.dma_start(ls_sb[:], lsr.ap())
            # Later stripes aren't needed until chunk 4+; hint them behind
            # the first gathers so they don't hog the DMA engines up front.
            with tc.tile_wait_until(0.02):
                for (s0, s1), part in zip(stripe_bounds[1:], idx_parts[1:]):
                    nc.sync.dma_start(part[:], idx16.ap()[:, s0:s1])
            els = finp.tile([P, 1], f32)
            nc.scalar.activation(els[:], ls_sb[:], mybir.ActivationFunctionType.Exp)
            ident = constp.tile([P, P], f32)
            make_identity(nc, ident[:])
            ident_bf = constp.tile([P, P], bf16)
            nc.vector.tensor_copy(out=ident_bf[:], in_=ident[:])

            sums_sb = miscp.tile([P, n_chunks * D], f32)
            sq_scr = finp.tile([P, D], f32)

            # ACT queue: the transposed caption embeds (finale-only input).
            cet_sb = [finp.tile([P, S], bf16, tag=f"cet{k}", name=f"cet{k}")
                      for k in range(k_tiles)]
            for k in range(k_tiles):
                nc.scalar.dma_start(cet_sb[k][:], cet.ap()[k * P : (k + 1) * P, :])

            # ---- main: gather + one-hot matmul segment reduction ----
            # Chunk-staged pipeline: all of chunk c's rows are gathered into
            # one chunk-wide buffer (2 in flight), then its one-hot matmuls
            # run as a single burst.  Keeps the PE busy in solid stretches
            # (no per-batch matmul/gather coupling) and lets gathers stream
            # at full descriptor rate.
            if need_memset:
                for _slot in range(gp_bufs):
                    g_init = gp.tile([P, batch_tiles, D], bf16, tag="g",
                                     name="g_init")
                    nc.vector.memset(g_init[:], 0)
            g_static = None
            if mode == "nogather":
                g_static = miscp.tile([P, batch_tiles, D], bf16)
                nc.vector.memset(g_static[:], 1.0)
            if mode == "nomm":
                nc.vector.memset(sums_sb[:], 1.0)
            # shared register holding the full batch count: only batches that
            # are partial on some core pay a per-batch reg_load.
            vreg_full = None
            if any(full_batches):
                vreg_full = nc.gpsimd.alloc_register()
                nc.gpsimd.reg_mov(vreg_full, batch_tiles * P)

            with tc.tile_pool(name="acc", bufs=acc_bufs, space="PSUM") as accp:
                for rep in range(main_reps):
                    for c in range(n_chunks):
                        gtiles = []
                        acc = None
                        if mode != "nomm":
                            acc = accp.tile([P, D], f32, tag="acc", name="acc")
                        done = 0
                        while done < tiles_per_chunk:
                            bt = min(batch_tiles, tiles_per_chunk - done)
                            if mode == "nogather":
                                done += bt
                                continue
                            g = gp.tile([P, batch_tiles, D], bf16,
                                        tag="g", name="g")
                            gtiles.append(g)
                            col0 = (c * tiles_per_chunk + done) * P // 16
                            nidx = bt * P
                            bidx = c * n_batches + done // batch_tiles
                            part = idx_parts[col0 // stripe_cols]
                            pc0 = col0 % stripe_cols
                            if full_batches[bidx] and bt == batch_tiles:
                                vreg = vreg_full
                            else:
                                vreg = nc.gpsimd.alloc_register()
                                nc.gpsimd.reg_load(
                                    vreg, cnt_sb[0:1, bidx : bidx + 1]
                                )
                            nc.gpsimd.dma_gather(
                                out_ap=g[:, :bt, :],
                                in_ap=adapter.ap(),
                                idxs_ap=part[:, pc0 : pc0 + nidx // 16],
                                num_idxs=nidx,
                                num_idxs_reg=vreg,
                                elem_size=D,
                                queue_num=bidx % 4,
                            )
                            if vreg is not vreg_full:
                                nc.gpsimd.free_register(vreg)
                            done += bt
                        if mode == "nomm":
                            continue
                        # matmuls layer-outer: each layer streams its one-hot
                        # batches sequentially; the gathered tiles stay live
                        # for the whole chunk (gp pool is sized for it).
                        n_mm = sum(layer_tiles)
                        mm_i = 0
                        for L in range(len(layer_tiles)):
                            lt = layer_tiles[L]
                            ct = min(cnt_tiles[L], lt)
                            t = 0
                            while t < lt:
                                if mode in ("noonehot", "nogather"):
                                    ohs = None
                                    w = min(batch_tiles, lt - t)
                                elif t < ct:
                                    # multiplicity>1 prefix: fused
                                    # (iota == seg) * cnt via TensorScalarPtr
                                    w = 1
                                    col = layer_off[L] + c * lt + t
                                    oh1 = ohp.tile([P, P], bf16, tag="oh",
                                                   name="oh")
                                    nc.vector.tensor_scalar(
                                        out=oh1[:],
                                        in0=iota_sb[:],
                                        scalar1=segf_sb[:, col : col + 1],
                                        scalar2=cntf_sb[:, col : col + 1],
                                        op0=mybir.AluOpType.is_equal,
                                        op1=mybir.AluOpType.mult,
                                    )
                                    ohs = [oh1[:, :]]
                                else:
                                    # batched transposed build: w tiles per
                                    # DVE op, all operands 2-byte packed
                                    w = min(batch_tiles, lt - t)
                                    col = layer_off[L] + c * lt + t
                                    ohb = ohbp.tile([P, P, batch_tiles], bf16,
                                                    tag="ohb", name="ohb")
                                    nc.vector.tensor_tensor(
                                        out=ohb[:, :, :w],
                                        in0=segfb_sb[:, col : col + w]
                                        .unsqueeze(1).to_broadcast([P, P, w]),
                                        in1=iotax_sb[:]
                                        .rearrange("p (j b) -> p j b",
                                                   b=batch_tiles)[:, :, :w],
                                        op=mybir.AluOpType.is_equal,
                                    )
                                    ohs = [ohb[:, :, j] for j in range(w)]
                                for j in range(w):
                                    tt = t + j
                                    if mode in ("noonehot", "nogather"):
                                        oh_ap = ident_bf[:, :]
                                    else:
                                        oh_ap = ohs[j]
                                    rhs = (g_static[:, 0, :]
                                           if mode == "nogather"
                                           else gtiles[tt // batch_tiles]
                                           [:, tt % batch_tiles, :])
                                    nc.tensor.matmul(
                                        acc[:],
                                        lhsT=oh_ap,
                                        rhs=rhs,
                                        start=(mm_i == 0),
                                        stop=(mm_i == n_mm - 1),
                                    )
                                    mm_i += 1
                                t += w
                        if rep == main_reps - 1:
                            if mode != "nomm":
                                nc.vector.tensor_copy(
                                    out=sums_sb[:, c * D : (c + 1) * D],
                                    in_=acc[:],
                                )
                            # stage this chunk's partial sums (ACT HWDGE queue
                            # so the SP queue stays free for other loads)
                            nc.scalar.dma_start(
                                cc_in.ap()[c * P : (c + 1) * P, :],
                                sums_sb[:, c * D : (c + 1) * D],
                            )
                            if c in (n_chunks // 2 - 1, n_chunks - 1):
                                h = 0 if c < n_chunks // 2 else 1
                                lo = h * half_rows
                                if single_core:
                                    nc.sync.dma_start(
                                        cc_out_h[h].ap(),
                                        cc_in.ap()[lo : lo + P, :],
                                    )
                                else:
                                    nc.gpsimd.collective_compute(
                                        "ReduceScatter",
                                        mybir.AluOpType.add,
                                        replica_groups=[list(range(n_cores))],
                                        ins=[cc_in.ap()[lo : lo + half_rows, :]],
                                        outs=[cc_out_h[h].ap()],
                                    )

            # ---- finale: per half-block normalize + logits rows ----
            # Pin the finale to the end of the schedule: without this the
            # tile scheduler hoists it into the middle of the main loop
            # (its collective input *can* be ready early), where it
            # head-of-line blocks the PE/DVE queues and stalls the gather
            # buffer recycling.
            finale_ctx = tc.tile_wait_until(0.3 * main_reps)
            finale_ctx.__enter__()
            pT = [finp.tile([P, out_rows], bf16, tag=f"pT{k}", name=f"pT{k}")
                  for k in range(k_tiles)]
            out_sb = [finp.tile([P, S], f32, tag=f"os{m}", name=f"os{m}")
                      for m in range(blk_tiles)]
            for m in range(blk_tiles):
                blk = finp.tile([P, D], f32, tag=f"blk{m}", name=f"blk{m}")
                nc.sync.dma_start(blk[:], cc_out_h[m].ap())
                rs_inv = finp.tile([P, 1], f32, tag=f"ri{m}", name=f"ri{m}")
                nc.scalar.activation(
                    sq_scr[:],
                    blk[:],
                    mybir.ActivationFunctionType.Square,
                    accum_out=rs_inv[:],
                )
                nc.scalar.sqrt(rs_inv[:], rs_inv[:])
                nc.vector.tensor_scalar_max(rs_inv[:], rs_inv[:], 1e-12)
                nc.vector.reciprocal(rs_inv[:], rs_inv[:])
                nc.vector.tensor_tensor(
                    out=rs_inv[:], in0=rs_inv[:], in1=els[:],
                    op=mybir.AluOpType.mult,
                )
                nc.vector.tensor_scalar(
                    out=blk[:],
                    in0=blk[:],
                    scalar1=rs_inv[:],
                    scalar2=None,
                    op0=mybir.AluOpType.mult,
                )
                for k in range(k_tiles):
                    t_ps = fpp.tile([P, P], f32, tag="tps", bufs=1)
                    nc.tensor.transpose(
                        t_ps[:], blk[:, k * P : (k + 1) * P], ident[:]
                    )
                    nc.vector.tensor_copy(
                        out=pT[k][:, m * P : (m + 1) * P], in_=t_ps[:]
                    )
                for n in range(n_tiles_out):
                    o_ps = fpp.tile([P, n_cols], f32, tag="ops", bufs=2)
                    for k in range(k_tiles):
                        nc.tensor.matmul(
                            o_ps[:],
                            lhsT=pT[k][:, m * P : (m + 1) * P],
                            rhs=cet_sb[k][:, n * n_cols : (n + 1) * n_cols],
                            start=(k == 0),
                            stop=(k == k_tiles - 1),
                        )
                    nc.vector.tensor_copy(
                        out=out_sb[m][:, n * n_cols : (n + 1) * n_cols],
                        in_=o_ps[:],
                    )
                nc.sync.dma_start(
                    out.ap()[m * P : (m + 1) * P, :], out_sb[m][:]
                )
            finale_ctx.__exit__(None, None, None)
    nc.compile()
    return nc


def _batch_counts(counts, tiles_per_chunk, batch_tiles, trim=True,
                  force_full_chunks=0):
    """Per-(core, chunk, batch) valid index counts, clamped to the batch.

    The first `force_full_chunks` chunks gather their full padded width
    (pad idx 0 fetches a real row) so the rotating chunk buffers are fully
    initialized before any trimmed chunk can expose stale SBUF bytes."""
    n_cores, n_chunks = counts.shape
    counts = counts.copy()
    if not trim:
        counts[:] = tiles_per_chunk * P
    counts[:, :force_full_chunks] = tiles_per_chunk * P
    n_batches = (tiles_per_chunk + batch_tiles - 1) // batch_tiles
    out = np.zeros((n_cores, n_chunks * n_batches), np.int32)
    for b in range(n_batches):
        start = b * batch_tiles * P
        width_tiles = min(batch_tiles, tiles_per_chunk - b * batch_tiles)
        cap = width_tiles * P
        vals = np.clip(counts - start, 0, cap)
        out[:, b::n_batches] = vals
    return out


def _make_in_maps(adapter_feats, caption_embed, logit_scale, idx16, layers,
                  n_cores, vox_per_core, counts=None, tiles_per_chunk=None,
                  batch_tiles=8, trim=True, force_full_chunks=0):
    import ml_dtypes

    bf = ml_dtypes.bfloat16
    af32 = np.asarray(adapter_feats, np.float32)
    af = np.ascontiguousarray(af32.astype(bf))  # [V, D] bf16
    ce_f32 = np.asarray(caption_embed, np.float32)
    ce_n = ce_f32 / np.clip(
        np.linalg.norm(ce_f32, axis=-1, keepdims=True), 1e-12, None
    )
    cet_np = np.ascontiguousarray(ce_n.T.astype(bf))
    ls = np.asarray(logit_scale, np.float32).reshape(-1)[0]
    ls_rep = np.full((P, 1), ls, np.float32)
    iota_mat = np.ascontiguousarray(
        np.broadcast_to(np.arange(P, dtype=np.float32), (P, P)).astype(bf)
    )
    iotax_mat = np.ascontiguousarray(
        np.broadcast_to(
            np.repeat(np.arange(P, dtype=np.float32), batch_tiles), (P, P * batch_tiles)
        ).astype(bf)
    )
    bc = _batch_counts(np.asarray(counts), tiles_per_chunk, batch_tiles,
                       trim=trim, force_full_chunks=force_full_chunks)
    in_maps = []
    for m in range(n_cores):
        in_maps.append(
            {
                "adapter": af[m * vox_per_core : (m + 1) * vox_per_core],
                "idx16": idx16[m],
                "segf": np.ascontiguousarray(
                    np.concatenate([sa for sa, _ in layers[m]], axis=1)),
                "cntf": np.ascontiguousarray(
                    np.concatenate([ca for _, ca in layers[m]], axis=1)),
                "segfb": np.ascontiguousarray(
                    np.concatenate([sa for sa, _ in layers[m]], axis=1).astype(bf)),
                "iota": iota_mat,
                "iotax": iotax_mat,
                "cet": cet_np,
                "lsr": ls_rep,
                "cnts": bc[m : m + 1],
            }
        )
    return in_maps


def _run(inputs_dict, n_cores, vox_per_core, D, S, batch_tiles=8, trace=False):
    from concourse.bass_utils import run_bass_kernel_spmd

    trim = True
    idx16, layers, tiles_per_chunk, layer_tiles, cnt_tiles, counts = _preprocess(
        inputs_dict["v2p_map"],
        inputs_dict["point_idx"],
        inputs_dict["seg_ids"],
        n_cores,
        vox_per_core,
        S // P,
        trim=True,
    )
    # a zero-valid-count gather would emit no descriptors and never fire its
    # completion semaphore; fall back to untrimmed padding in that case
    if _batch_counts(counts, tiles_per_chunk, batch_tiles, trim=True).min() == 0:
        trim = False
        idx16, layers, tiles_per_chunk, layer_tiles, cnt_tiles, counts = _preprocess(
            inputs_dict["v2p_map"],
            inputs_dict["point_idx"],
            inputs_dict["seg_ids"],
            n_cores,
            vox_per_core,
            S // P,
            trim=False,
        )
    # The first gp_bufs chunks gather untrimmed (pad idx 0 fetches row 0),
    # so every rotating chunk buffer is fully written with finite data
    # before any trimmed chunk can expose stale SBUF bytes.
    import os as _os
    gp_bufs = int(_os.environ.get("GP_BUFS", "13"))
    # stale-SBUF guard: the rotating gather buffers are only safe without
    # an init memset if the first gp_bufs batches are full everywhere
    need_memset = bool(trim) and counts.min() < gp_bufs * batch_tiles * P
    bc = _batch_counts(counts, tiles_per_chunk, batch_tiles, trim=trim)
    full_batches = (bc.min(axis=0) == _batch_counts(
        np.full_like(counts, tiles_per_chunk * P), tiles_per_chunk,
        batch_tiles).min(axis=0)).tolist()
    if _os.environ.get("NO_SHARED_REG"):
        full_batches = [False] * len(full_batches)
    nc = _build_nc(tiles_per_chunk, vox_per_core, D, S, n_cores, layer_tiles,
                   cnt_tiles=cnt_tiles, batch_tiles=batch_tiles,
                   gp_bufs=gp_bufs, full_batches=full_batches,
                   need_memset=need_memset)
    in_maps = _make_in_maps(
        inputs_dict["adapter_feats"],
        inputs_dict["caption_embed"],
        inputs_dict["logit_scale"],
        idx16,
        layers,
        n_cores,
        vox_per_core,
        counts=counts,
        tiles_per_chunk=tiles_per_chunk,
        batch_tiles=batch_tiles,
        trim=trim,
    )
    res = run_bass_kernel_spmd(
        nc, in_maps, core_ids=list(range(n_cores)), trace=trace
    )
    blocks = [res.results[m]["logits_block"] for m in range(n_cores)]
    return _assemble(blocks, S, n_cores), res


def _assemble(blocks, S, n_cores):
    """Core r's output block holds segment rows for chunk r (tile 0) and
    chunk n_cores+r (tile 1)."""
    half = S // 2
    full = np.empty((S, blocks[0].shape[1]), blocks[0].dtype)
    for r in range(n_cores):
        full[r * P : (r + 1) * P] = blocks[r][:P]
        full[half + r * P : half + (r + 1) * P] = blocks[r][P : 2 * P]
    return full


def kernel(adapter_feats, caption_embed, logit_scale, v2p_map, point_idx,
           seg_ids, num_segments=S_FULL, **_):
    logits, _res = _run(
        {
            "adapter_feats": adapter_feats,
            "caption_embed": caption_embed,
            "logit_scale": logit_scale,
            "v2p_map": v2p_map,
            "point_idx": point_idx,
            "seg_ids": seg_ids,
        },
        N_CORES,
        N_VOX // N_CORES,
        D_FULL,
        S_FULL,
    )
    return logits

